# revision 2
# baseline (speedup 1.0000x reference)
"""Trainium2 Bass kernel for nn_BinTreeNetwork (binary-tree MLP expansion).

Strategy
--------
The reference is a 21-level binary-tree expansion ending at a (2,)^21 x 32
fp32 output (256 MB). Everything is linear; in flat memory terms each
iteration maps state rows (L, R: M x 2) and accumulator (out: M x 32) to
2M rows via

  res = [L[:M/2]; R[:M/2]; L[M/2:]; R[M/2:]]          (2M x 2)
  out' = [out + C[:M]; out + C[M:]],  C = res @ Wo_i.T
  L', R' = res @ Wl_i.T + bl_i,  res @ Wr_i.T + br_i

The row-index bit structure makes a mod-8 row sharding communication-free:
core q owns rows ≡ q (mod 8), and the recursion restricted to those rows
has the identical flat form. The state path is tiny (2 floats per row), so
the host computes it exactly in fp32 numpy through level 20, shipping each
core its o-accumulator at level 20 (16 MB) and the last level's res planes
(2 MB). The device performs only the bandwidth-heavy final expansion:

  PSUM  = blockdiag(Wo_20).T @ res20-chunk   (TensorE, float32r, K=8)
  out   = PSUM + out_bias + o20[wrapped]     (VectorE fused scalar_tensor_tensor)
  DMA out-chunk -> DRAM                      (streamed, never materialized)

o tiles use a "mod-4 stacked plane" layout [128, M/4]: partition
32*(row%4)+plane, column row//4, which makes the tree-doubling broadcast a
pure column-slice operation, keeps every engine op at full 128-partition
width, and makes all DMAs fully contiguous. The o-accumulator path stays
exact fp32 end to end; only the per-level C contributions go through the
PE's float32r multiplies (~1e-4 relative, measured ~4e-5 absmax-rel).
"""
import numpy as np
from contextlib import ExitStack

import concourse.bass as bass
import concourse.bacc as bacc
import concourse.mybir as mybir
import concourse.tile as tile
from concourse.bass_utils import run_bass_kernel_spmd

T = 21
L0 = 20
CHUNK = 1536
MM_DT = mybir.dt.float32r
F32 = mybir.dt.float32

_CACHE = {}


# ---------------- host-side exact precompute ----------------

def _host_precompute(inputs):
    x = inputs["x"].astype(np.float32)
    L = (x @ inputs["in_left_layer"].T + inputs["in_left_bias"]).reshape(1, 2).astype(np.float32)
    R = (x @ inputs["in_right_layer"].T + inputs["in_right_bias"]).reshape(1, 2).astype(np.float32)
    out = (x @ inputs["out_layer0"].T).reshape(1, 32).astype(np.float32)
    res_levels = []
    o_L0 = None
    for i in range(T):
        M = L.shape[0]
        if i == L0:
            o_L0 = out
        if M == 1:
            res = np.array([[L[0, 0], R[0, 0]], [L[0, 1], R[0, 1]]], np.float32)
        else:
            res = np.concatenate([L[: M // 2], R[: M // 2], L[M // 2 :], R[M // 2 :]], axis=0)
        if i >= L0:
            res_levels.append(res)
        if i < L0:
            C = res @ inputs["out_layers"][i].T
            out = np.concatenate([out + C[:M], out + C[M:]], axis=0)
        if i < T - 1:  # last level's L/R states are unused
            L = res @ inputs["tree_left_layers"][i].T + inputs["tree_left_biases"][i]
            R = res @ inputs["tree_right_layers"][i].T + inputs["tree_right_biases"][i]
    return o_L0, res_levels


def _pack_o_mod4(o_rows):
    M = o_rows.shape[0]
    return np.ascontiguousarray(
        o_rows.reshape(M // 4, 4, 32).transpose(1, 2, 0).reshape(128, M // 4), np.float32)


def _unpack_o_mod4(t):
    Mc = t.shape[1]
    return np.ascontiguousarray(
        t.reshape(4, 32, Mc).transpose(2, 0, 1).reshape(4 * Mc, 32), np.float32)


def _pack_res8(res):
    m2 = res.shape[0]
    cols = m2 // 4
    return np.ascontiguousarray(
        res.reshape(cols, 4, 2).transpose(1, 2, 0).reshape(8, cols), np.float32)


def _make_lhsT(Wo):
    t = np.zeros((8, 128), np.float32)
    for b in range(4):
        for f in range(2):
            t[2 * b + f, 32 * b: 32 * (b + 1)] = Wo[:, f]
    return t


# ---------------- device program ----------------

def _level_rows():
    return [2 ** (i - 3) for i in range(L0, T)]


def _build_nc():
    Ms = _level_rows()
    nlev = len(Ms)
    OUTC = Ms[-1] // 2

    nc = bacc.Bacc("TRN2", target_bir_lowering=False, debug=False,
                   enable_asserts=True, num_devices=8)

    o_init_d = nc.dram_tensor("o_init", [128, Ms[0] // 4], F32, kind="ExternalInput").ap()
    res_d = [nc.dram_tensor(f"res{li}", [8, M // 2], MM_DT, kind="ExternalInput").ap()
             for li, M in enumerate(Ms)]
    wc_d = nc.dram_tensor("wc", [8, nlev * 128], MM_DT, kind="ExternalInput").ap()
    obias_d = nc.dram_tensor("obias", [128, 1], F32, kind="ExternalInput").ap()
    out_d = nc.dram_tensor("out", [128, OUTC], F32, kind="ExternalOutput").ap()

    with tile.TileContext(nc, trace_sim=False) as tc:
        ctx = ExitStack()
        with ctx:
            const_pool = ctx.enter_context(tc.tile_pool(name="consts", bufs=1))
            ost_pool = ctx.enter_context(tc.tile_pool(name="ostate", bufs=1))
            res_pool = ctx.enter_context(tc.tile_pool(name="resc", bufs=4))
            outc_pool = ctx.enter_context(tc.tile_pool(name="outc", bufs=4))
            psum_pool = ctx.enter_context(tc.tile_pool(name="ps", bufs=2, space="PSUM"))

            wc_sb = const_pool.tile([8, nlev * 128], MM_DT, name="wc_sb")
            nc.scalar.dma_start(out=wc_sb[:], in_=wc_d[:])
            obias_sb = const_pool.tile([128, 1], F32, name="obias_sb")
            nc.scalar.dma_start(out=obias_sb[:], in_=obias_d[:])

            # o state arrives via the (otherwise idle) SWDGE queue; slices are
            # emitted staggered with the chunk loop below so the first res
            # chunks don't queue behind 16 MB of o-state on the SDMA engines.
            o_prev = ost_pool.tile([128, Ms[0] // 4], F32, name="o_init_sb")
            C0 = Ms[0] // 4
            OSLICE = 2048

            def mm_chunk(li, c0, c1, ptile, dma_eng=None):
                cw = c1 - c0
                rt = res_pool.tile([8, cw], MM_DT, name=f"rc{li}_{c0}", tag="resc")
                (dma_eng or nc.scalar).dma_start(out=rt[:, :cw], in_=res_d[li][:, c0:c1])
                lhsT = wc_sb[:, li * 128:(li + 1) * 128]
                s = 0
                while s < cw:
                    e = min(s + 512, cw)
                    nc.tensor.matmul(ptile[:, s:e], lhsT, rt[:, s:e],
                                     start=True, stop=True)
                    s = e

            colsB, halfB = Ms[-1] // 2, Ms[-1] // 4
            c0 = 0
            osl = 0
            nch = 0
            while c0 < colsB:
                c1 = min(c0 + CHUNK, halfB if c0 < halfB else colsB)
                cw = c1 - c0
                need = min(C0, (c0 % halfB) + cw + 3 * OSLICE)
                while osl < need:
                    oe = min(osl + OSLICE, C0)
                    nc.gpsimd.dma_start(out=o_prev[:, osl:oe], in_=o_init_d[:, osl:oe])
                    osl = oe
                ptB = psum_pool.tile([128, cw], F32, name=f"pB_{c0}", tag="ps")
                mm_chunk(nlev - 1, c0, c1, ptB,
                         dma_eng=nc.sync if nch < 3 else None)
                nch += 1
                ot = outc_pool.tile([128, cw], F32, name=f"ot_{c0}", tag="outc")
                nc.vector.scalar_tensor_tensor(
                    ot[:, :cw], ptB[:, :cw], obias_sb[:],
                    o_prev[:, c0 % halfB: c0 % halfB + cw],
                    mybir.AluOpType.add, mybir.AluOpType.add)
                nc.sync.dma_start(out=out_d[:, c0:c0 + cw], in_=ot[:, :cw])
                c0 = c1

    nc.compile()
    return nc


# ---------------- entry point ----------------

def prepare(inputs):
    inputs = {k: np.asarray(v) for k, v in inputs.items()}
    o_L0, res_levels = _host_precompute(inputs)

    if "nc" not in _CACHE:
        _CACHE["nc"] = _build_nc()
    nc = _CACHE["nc"]

    nlev = T - L0
    wc = np.ascontiguousarray(np.concatenate(
        [_make_lhsT(np.asarray(inputs["out_layers"][L0 + li], np.float32))
         for li in range(nlev)], axis=1))
    obias = np.ascontiguousarray(
        np.tile(np.asarray(inputs["out_bias"], np.float32), 4).reshape(128, 1))

    in_maps = []
    for q in range(8):
        m = {"wc": wc, "obias": obias, "o_init": _pack_o_mod4(o_L0[q::8])}
        for li in range(nlev):
            m[f"res{li}"] = _pack_res8(np.ascontiguousarray(res_levels[li][q::8]))
        in_maps.append(m)
    return nc, in_maps


def assemble(results):
    full = np.empty((2 ** T, 32), np.float32)
    for q in range(8):
        full[q::8] = _unpack_o_mod4(results[q]["out"])
    return full.reshape((2,) * T + (32,))


def kernel(**inputs):
    nc, in_maps = prepare(inputs)
    res = run_bass_kernel_spmd(nc, in_maps, list(range(8)))
    return assemble(res.results)



# revision 3
# speedup vs baseline: 1.0875x; 1.0875x over previous
"""Trainium2 Bass kernel for nn_BinTreeNetwork (binary-tree MLP expansion).

Strategy (v2)
-------------
The reference is a 21-level binary-tree expansion ending at a (2,)^21 x 32
fp32 output (256 MB). Everything is linear; in flat memory terms each
iteration maps state rows (L, R: M x 2) and accumulator (out: M x 32) to
2M rows via

  res = [L[:M/2]; R[:M/2]; L[M/2:]; R[M/2:]]          (2M x 2)
  out' = [out + C[:M]; out + C[M:]],  C = res @ Wo_i.T
  L', R' = res @ Wl_i.T + bl_i,  res @ Wr_i.T + br_i

Mod-8 row sharding is communication-free (core q owns rows ≡ q mod 8) and
preserves the flat form. The host computes the tiny L/R state path exactly
in fp32 and ships only the *res* planes (levels 14..20, bf16) plus the
level-14 o-accumulator (fp32, 256 KB/core). All bandwidth-heavy work runs
on-device, so per-core HBM traffic is ~36 MB (vs 52 MB for the
ship-o20-from-host variant): the 33.5 MB output write IS the roofline.

Device program (per core):
  1. o-tree build: levels 14..18 in fp32 SBUF.  C_i = blockdiag(Wo_i).T @
     res_i (TensorE, bf16, K=8), then o_{i+1} = C_i + o_i[wrapped] via
     VectorE scalar_tensor_tensor. Tiny: ~32k matmul cols, ~8 MB of DVE.
  2. Final pass, streamed in 2048-col chunks: levels 19+20 are folded into
     ONE K=16 matmul — rhs partitions 0-7 carry res20, partitions 8-15
     carry res19 pre-replicated x2 by the host (wrap period 16384 cols),
     so the 16.8 MB o_20 never exists anywhere.  PSUM then holds
     C_20 + C_19; out = PSUM + out_bias + o_19[wrapped] (VectorE fused),
     DMA chunk -> DRAM.

o/out tiles use the "mod-4 stacked plane" layout [128, M/4]: partition
32*(row%4)+plane, column row//4, making the tree-doubling broadcast a pure
column-slice operation and all DMAs fully contiguous. The o-accumulator
path stays fp32 end to end; only res values and Wo weights are bf16
(measured ~1e-3 norm-rel, tolerance 2e-2).
"""
import numpy as np
import ml_dtypes
from contextlib import ExitStack

import concourse.bass as bass
import concourse.bacc as bacc
import concourse.mybir as mybir
import concourse.tile as tile
from concourse.bass_utils import run_bass_kernel_spmd

T = 21
L0 = 14          # host ships o at this level; device does levels 14..20
NLOW = 5         # levels 14..18 build the o-tree on device
FCOLS = 65536    # final out cols per core ( 2^21/8 rows / 4 per col )
O19C = 16384     # o_19 cols per core
CHUNK = 2048
BF16 = mybir.dt.bfloat16
F32 = mybir.dt.float32
RCOLS = [1 << (10 + li) for li in range(NLOW)]   # C_i cols, i=14..18
RLOW_OFF = np.cumsum([0] + RCOLS).tolist()       # col offsets in reslow
RLOW_TOT = RLOW_OFF[-1]                          # 31744

_CACHE = {}


# ---------------- host-side exact precompute ----------------

def _host_precompute(inputs):
    x = inputs["x"].astype(np.float32)
    L = (x @ inputs["in_left_layer"].T + inputs["in_left_bias"]).reshape(1, 2).astype(np.float32)
    R = (x @ inputs["in_right_layer"].T + inputs["in_right_bias"]).reshape(1, 2).astype(np.float32)
    out = (x @ inputs["out_layer0"].T).reshape(1, 32).astype(np.float32)
    res_levels = []
    o_L0 = None
    for i in range(T):
        M = L.shape[0]
        if i == L0:
            o_L0 = out
        if M == 1:
            res = np.array([[L[0, 0], R[0, 0]], [L[0, 1], R[0, 1]]], np.float32)
        else:
            res = np.concatenate([L[: M // 2], R[: M // 2], L[M // 2 :], R[M // 2 :]], axis=0)
        if i >= L0:
            res_levels.append(res)
        if i < L0:
            C = res @ inputs["out_layers"][i].T
            out = np.concatenate([out + C[:M], out + C[M:]], axis=0)
        if i < T - 1:  # last level's L/R states are unused
            L = res @ inputs["tree_left_layers"][i].T + inputs["tree_left_biases"][i]
            R = res @ inputs["tree_right_layers"][i].T + inputs["tree_right_biases"][i]
    return o_L0, res_levels


def _pack_o_mod4(o_rows):
    M = o_rows.shape[0]
    return np.ascontiguousarray(
        o_rows.reshape(M // 4, 4, 32).transpose(1, 2, 0).reshape(128, M // 4), np.float32)


def _unpack_o_mod4(t):
    Mc = t.shape[1]
    return np.ascontiguousarray(
        t.reshape(4, 32, Mc).transpose(2, 0, 1).reshape(4 * Mc, 32), np.float32)


def _pack_res8(res):
    m2 = res.shape[0]
    cols = m2 // 4
    return np.ascontiguousarray(
        res.reshape(cols, 4, 2).transpose(1, 2, 0).reshape(8, cols), np.float32)


def _make_lhsT(Wo):
    t = np.zeros((8, 128), np.float32)
    for b in range(4):
        for f in range(2):
            t[2 * b + f, 32 * b: 32 * (b + 1)] = Wo[:, f]
    return t


# ---------------- device program ----------------

def _build_nc():
    nc = bacc.Bacc("TRN2", target_bir_lowering=False, debug=False,
                   enable_asserts=True, num_devices=8)

    rhsf_d = nc.dram_tensor("rhsf", [16, FCOLS], BF16, kind="ExternalInput").ap()
    reslow_d = nc.dram_tensor("reslow", [8, RLOW_TOT], BF16, kind="ExternalInput").ap()
    o14_d = nc.dram_tensor("o14", [128, 512], F32, kind="ExternalInput").ap()
    wfin_d = nc.dram_tensor("wfin", [16, 128], BF16, kind="ExternalInput").ap()
    wlow_d = nc.dram_tensor("wlow", [8, NLOW * 128], BF16, kind="ExternalInput").ap()
    obias_d = nc.dram_tensor("obias", [128, 1], F32, kind="ExternalInput").ap()
    out_d = nc.dram_tensor("out", [128, FCOLS], F32, kind="ExternalOutput").ap()

    with tile.TileContext(nc, trace_sim=False) as tc:
        ctx = ExitStack()
        with ctx:
            const_pool = ctx.enter_context(tc.tile_pool(name="consts", bufs=1))
            otree_pool = ctx.enter_context(tc.tile_pool(name="otree", bufs=1))
            lres_pool = ctx.enter_context(tc.tile_pool(name="lres", bufs=3))
            rhs_pool = ctx.enter_context(tc.tile_pool(name="rhsc", bufs=4))
            outc_pool = ctx.enter_context(tc.tile_pool(name="outc", bufs=4))
            psum_pool = ctx.enter_context(tc.tile_pool(name="ps", bufs=2, space="PSUM"))

            wfin_sb = const_pool.tile([16, 128], BF16, name="wfin_sb")
            nc.scalar.dma_start(out=wfin_sb[:], in_=wfin_d[:])
            wlow_sb = const_pool.tile([8, NLOW * 128], BF16, name="wlow_sb")
            nc.scalar.dma_start(out=wlow_sb[:], in_=wlow_d[:])
            obias_sb = const_pool.tile([128, 1], F32, name="obias_sb")
            nc.scalar.dma_start(out=obias_sb[:], in_=obias_d[:])

            # o-tree tiles, fp32: o_14 (512 cols) .. o_19 (16384 cols)
            ot = [otree_pool.tile([128, 512 << li], F32, name=f"o{14 + li}_sb")
                  for li in range(NLOW + 1)]
            nc.gpsimd.dma_start(out=ot[0][:], in_=o14_d[:])

            # ---- phase 1: o-tree levels 14..18 ----
            for li in range(NLOW):
                RC = RCOLS[li]          # C_i cols
                OIN = RC // 2           # o_i cols
                lhs = wlow_sb[:, li * 128:(li + 1) * 128]
                o_in, o_out = ot[li], ot[li + 1]
                for a in range(0, RC, CHUNK):
                    w = min(CHUNK, RC - a)
                    rt = lres_pool.tile([8, w], BF16, name=f"lr{li}_{a}", tag="lres")
                    nc.gpsimd.dma_start(
                        out=rt[:, :w], in_=reslow_d[:, RLOW_OFF[li] + a: RLOW_OFF[li] + a + w])
                    pt = psum_pool.tile([128, w], F32, name=f"pl{li}_{a}", tag="ps")
                    s = 0
                    while s < w:
                        e = min(s + 512, w)
                        nc.tensor.matmul(pt[:, s:e], lhs, rt[:, s:e],
                                         start=True, stop=True)
                        s = e
                    # o_out[:, a:a+w] = pt + o_in[:, (a..) mod OIN]
                    b = 0
                    while b < w:
                        bw = min(OIN, w - b)
                        src = (a + b) % OIN
                        nc.vector.scalar_tensor_tensor(
                            o_out[:, a + b: a + b + bw], pt[:, b: b + bw], 0.0,
                            o_in[:, src: src + bw],
                            mybir.AluOpType.add, mybir.AluOpType.add)
                        b += bw

            # ---- phase 2: final pass, levels 19+20 fused (K=16) ----
            o19 = ot[NLOW]
            for c in range(0, FCOLS, CHUNK):
                rt = rhs_pool.tile([16, CHUNK], BF16, name=f"rf{c}", tag="rhsc")
                nc.scalar.dma_start(out=rt[:], in_=rhsf_d[:, c: c + CHUNK])
                pt = psum_pool.tile([128, CHUNK], F32, name=f"pf_{c}", tag="ps")
                s = 0
                while s < CHUNK:
                    e = s + 512
                    nc.tensor.matmul(pt[:, s:e], wfin_sb[:], rt[:, s:e],
                                     start=True, stop=True)
                    s = e
                otile = outc_pool.tile([128, CHUNK], F32, name=f"ot_{c}", tag="outc")
                src = c % O19C
                nc.vector.scalar_tensor_tensor(
                    otile[:], pt[:], obias_sb[:],
                    o19[:, src: src + CHUNK],
                    mybir.AluOpType.add, mybir.AluOpType.add)
                nc.sync.dma_start(out=out_d[:, c: c + CHUNK], in_=otile[:])

    nc.compile()
    return nc


# ---------------- entry point ----------------

def prepare(inputs):
    inputs = {k: np.asarray(v) for k, v in inputs.items()}
    o_L0, res_levels = _host_precompute(inputs)

    if "nc" not in _CACHE:
        _CACHE["nc"] = _build_nc()
    nc = _CACHE["nc"]

    bf = lambda a: np.ascontiguousarray(a).astype(ml_dtypes.bfloat16)
    wlow = bf(np.concatenate(
        [_make_lhsT(np.asarray(inputs["out_layers"][L0 + li], np.float32))
         for li in range(NLOW)], axis=1))
    wfin = bf(np.concatenate(
        [_make_lhsT(np.asarray(inputs["out_layers"][20], np.float32)),
         _make_lhsT(np.asarray(inputs["out_layers"][19], np.float32))], axis=0))
    obias = np.ascontiguousarray(
        np.tile(np.asarray(inputs["out_bias"], np.float32), 4).reshape(128, 1))

    in_maps = []
    for q in range(8):
        packed = [_pack_res8(np.ascontiguousarray(r[q::8])) for r in res_levels]
        reslow = bf(np.concatenate(packed[:NLOW], axis=1))
        rhsf = bf(np.concatenate(
            [packed[6], np.tile(packed[5], (1, 2))], axis=0))
        in_maps.append({
            "rhsf": rhsf, "reslow": reslow,
            "o14": _pack_o_mod4(o_L0[q::8]),
            "wfin": wfin, "wlow": wlow, "obias": obias,
        })
    return nc, in_maps


def assemble(results):
    full = np.empty((2 ** T, 32), np.float32)
    for q in range(8):
        full[q::8] = _unpack_o_mod4(results[q]["out"])
    return full.reshape((2,) * T + (32,))


def kernel(**inputs):
    nc, in_maps = prepare(inputs)
    res = run_bass_kernel_spmd(nc, in_maps, list(range(8)))
    return assemble(res.results)


# revision 5
# speedup vs baseline: 1.1442x; 1.0522x over previous
"""Trainium2 Bass kernel for nn_BinTreeNetwork (binary-tree MLP expansion).

Strategy (v3)
-------------
The reference is a 21-level binary-tree expansion ending at a (2,)^21 x 32
fp32 output (256 MB). Everything is linear; in flat memory terms each
iteration maps state rows (L, R: M x 2) and accumulator (out: M x 32) to
2M rows via

  res = [L[:M/2]; R[:M/2]; L[M/2:]; R[M/2:]]          (2M x 2)
  out' = [out + C[:M]; out + C[M:]],  C = res @ Wo_i.T
  L', R' = res @ Wl_i.T + bl_i,  res @ Wr_i.T + br_i

Mod-8 row sharding is communication-free (core q owns rows ≡ q mod 8) and
preserves the flat form. The host computes the tiny L/R state path exactly
in fp32 and ships only *res* planes (levels 14..20, bf16, wrap-replicated
where needed) plus the level-14 o-accumulator (fp32, 256 KB/core), so
per-core HBM traffic is ~37 MB and the 33.5 MB output write IS the
roofline (vs 52 MB/core for a ship-o20 variant).

Key devices tricks:
  * K-folding: because a matmul's cost is ~1 column/cycle regardless of K,
    multiple tree levels are contracted in ONE matmul by stacking their
    res planes on the K axis (lower levels pre-replicated by the host to
    match the wrap period).  Final pass: levels 18+19+20 as K=24.
    Internal: (14,15)->o16 and (16,17)->o18 as K=16.  The o-tree touches
    only 3 small fp32 SBUF tiles; o_19/o_20 never exist anywhere.
  * PE row-group rotation: successive chunks place weights+rhs at
    partition base 32g (g = chunk mod 4) with tile_position=(32g, 0), so
    up to 4 matmuls run concurrently in disjoint 32-row PE sub-arrays and
    per-chunk LDWEIGHTS is pulled ahead instead of serializing.
  * The fused out = PSUM + out_bias + o_18[wrapped] pass alternates
    between VectorE and GpSimdE per chunk - either alone would be the
    critical path.

o/out tiles use the "mod-4 stacked plane" layout [128, M/4]: partition
32*(row%4)+plane, column row//4, making the tree-doubling broadcast a pure
column-slice operation and all DMAs fully contiguous. The o-accumulator
path stays fp32 end to end; only res values and Wo weights are bf16
(measured ~1e-3 norm-rel, tolerance 2e-2).
"""
import numpy as np
import ml_dtypes
from contextlib import ExitStack

import concourse.bass as bass
import concourse.bacc as bacc
import concourse.mybir as mybir
import concourse.tile as tile
from concourse.bass_utils import run_bass_kernel_spmd

T = 21
L0 = 14          # host ships o at this level; device does levels 14..20
FCOLS = 65536    # final out cols per core ( 2^21/8 rows / 4 per col )
CH = 1024
BF16 = mybir.dt.bfloat16
F32 = mybir.dt.float32
ADD = mybir.AluOpType.add

# internal K=16 folds: (lo, hi) level pairs -> o_{hi+1}; final fold 18+19+20
FOLD_A_COLS = 2048    # C_15 cols; produces o_16
FOLD_B_COLS = 8192    # C_17 cols; produces o_18
O18C = 8192           # o_18 cols

_CACHE = {}


# ---------------- host-side exact precompute ----------------

def _host_precompute(inputs):
    x = inputs["x"].astype(np.float32)
    L = (x @ inputs["in_left_layer"].T + inputs["in_left_bias"]).reshape(1, 2).astype(np.float32)
    R = (x @ inputs["in_right_layer"].T + inputs["in_right_bias"]).reshape(1, 2).astype(np.float32)
    out = (x @ inputs["out_layer0"].T).reshape(1, 32).astype(np.float32)
    res_levels = []
    o_L0 = None
    for i in range(T):
        M = L.shape[0]
        if i == L0:
            o_L0 = out
        if M == 1:
            res = np.array([[L[0, 0], R[0, 0]], [L[0, 1], R[0, 1]]], np.float32)
        else:
            res = np.concatenate([L[: M // 2], R[: M // 2], L[M // 2 :], R[M // 2 :]], axis=0)
        if i >= L0:
            res_levels.append(res)
        if i < L0:
            C = res @ inputs["out_layers"][i].T
            out = np.concatenate([out + C[:M], out + C[M:]], axis=0)
        if i < T - 1:  # last level's L/R states are unused
            L = res @ inputs["tree_left_layers"][i].T + inputs["tree_left_biases"][i]
            R = res @ inputs["tree_right_layers"][i].T + inputs["tree_right_biases"][i]
    return o_L0, res_levels


def _pack_o_mod4(o_rows):
    M = o_rows.shape[0]
    return np.ascontiguousarray(
        o_rows.reshape(M // 4, 4, 32).transpose(1, 2, 0).reshape(128, M // 4), np.float32)


def _unpack_o_mod4(t):
    Mc = t.shape[1]
    return np.ascontiguousarray(
        t.reshape(4, 32, Mc).transpose(2, 0, 1).reshape(4 * Mc, 32), np.float32)


def _pack_res8(res):
    m2 = res.shape[0]
    cols = m2 // 4
    return np.ascontiguousarray(
        res.reshape(cols, 4, 2).transpose(1, 2, 0).reshape(8, cols), np.float32)


def _make_lhsT(Wo):
    t = np.zeros((8, 128), np.float32)
    for b in range(4):
        for f in range(2):
            t[2 * b + f, 32 * b: 32 * (b + 1)] = Wo[:, f]
    return t


# ---------------- device program ----------------

def _build_nc():
    nc = bacc.Bacc("TRN2", target_bir_lowering=False, debug=False,
                   enable_asserts=True, num_devices=8)

    rhsf_d = nc.dram_tensor("rhsf", [24, FCOLS], BF16, kind="ExternalInput").ap()
    rhsab_d = nc.dram_tensor("rhsab", [16, FOLD_A_COLS + FOLD_B_COLS], BF16,
                             kind="ExternalInput").ap()
    o14_d = nc.dram_tensor("o14", [128, 512], F32, kind="ExternalInput").ap()
    w4_d = nc.dram_tensor("w4", [128, 3 * 128], BF16, kind="ExternalInput").ap()
    obias_d = nc.dram_tensor("obias", [128, 1], F32, kind="ExternalInput").ap()
    out_d = nc.dram_tensor("out", [128, FCOLS], F32, kind="ExternalOutput").ap()

    with tile.TileContext(nc, trace_sim=False) as tc:
        ctx = ExitStack()
        with ctx:
            const_pool = ctx.enter_context(tc.tile_pool(name="consts", bufs=1))
            otree_pool = ctx.enter_context(tc.tile_pool(name="otree", bufs=1))
            rhs_pool = ctx.enter_context(tc.tile_pool(name="rhsc", bufs=6))
            outc_pool = ctx.enter_context(tc.tile_pool(name="outc", bufs=4))
            psum_pool = ctx.enter_context(tc.tile_pool(name="ps", bufs=4, space="PSUM"))

            w4_sb = const_pool.tile([128, 3 * 128], BF16, name="w4_sb")
            nc.scalar.dma_start(out=w4_sb[:], in_=w4_d[:])
            obias_sb = const_pool.tile([128, 1], F32, name="obias_sb")
            nc.scalar.dma_start(out=obias_sb[:], in_=obias_d[:])

            o14 = otree_pool.tile([128, 512], F32, name="o14_sb")
            o16 = otree_pool.tile([128, 2048], F32, name="o16_sb")
            o18 = otree_pool.tile([128, O18C], F32, name="o18_sb")
            nc.scalar.dma_start(out=o14[:], in_=o14_d[:])

            state = {"g": 0, "par": 0}

            def fold_chunk(a, src_d, src_off, K, wcol, o_in, oin_cols,
                           o_out=None, final=False):
                g = state["g"] % 4
                state["g"] += 1
                p0 = 32 * g
                rt = rhs_pool.tile([128, CH], BF16, name=f"r{state['g']}", tag="rhsc")
                nc.scalar.dma_start(out=rt[p0:p0 + K, :],
                                    in_=src_d[:, src_off + a: src_off + a + CH])
                pt = psum_pool.tile([128, CH], F32, name=f"p{state['g']}", tag="ps")
                for s in (0, 512):
                    nc.tensor.matmul(pt[:, s:s + 512],
                                     w4_sb[p0:p0 + K, wcol:wcol + 128],
                                     rt[p0:p0 + K, s:s + 512],
                                     start=True, stop=True, tile_position=(p0, 0))
                if final:
                    otile = outc_pool.tile([128, CH], F32, name=f"o{a}", tag="outc")
                    src = a % oin_cols
                    nc.vector.scalar_tensor_tensor(otile[:], pt[:], obias_sb[:],
                                                   o_in[:, src: src + CH], ADD, ADD)
                    nc.sync.dma_start(out=out_d[:, a: a + CH], in_=otile[:])
                else:
                    b = 0
                    while b < CH:
                        bw = min(oin_cols, CH - b)
                        src = (a + b) % oin_cols
                        nc.vector.scalar_tensor_tensor(
                            o_out[:, a + b: a + b + bw], pt[:, b: b + bw], 0.0,
                            o_in[:, src: src + bw], ADD, ADD)
                        b += bw

            # fold A: levels 14+15 -> o_16   (K=16, weights at w4 cols 0:128)
            for a in range(0, FOLD_A_COLS, CH):
                fold_chunk(a, rhsab_d, 0, 16, 0, o14, 512, o_out=o16)
            # fold B: levels 16+17 -> o_18   (K=16, weights at w4 cols 128:256)
            for a in range(0, FOLD_B_COLS, CH):
                fold_chunk(a, rhsab_d, FOLD_A_COLS, 16, 128, o16, 2048, o_out=o18)
            # final: levels 18+19+20 -> out  (K=24, weights at w4 cols 256:384)
            for a in range(0, FCOLS, CH):
                fold_chunk(a, rhsf_d, 0, 24, 256, o18, O18C, final=True)

    nc.compile()
    return nc


# ---------------- entry point ----------------

def prepare(inputs):
    inputs = {k: np.asarray(v) for k, v in inputs.items()}
    o_L0, res_levels = _host_precompute(inputs)

    if "nc" not in _CACHE:
        _CACHE["nc"] = _build_nc()
    nc = _CACHE["nc"]

    bf = lambda a: np.ascontiguousarray(a).astype(ml_dtypes.bfloat16)
    lhs = [_make_lhsT(np.asarray(inputs["out_layers"][L0 + li], np.float32))
           for li in range(7)]   # levels 14..20 -> lhs[0..6]
    w4 = np.zeros((128, 3 * 128), np.float32)
    for g in range(4):
        w4[32 * g: 32 * g + 8, 0:128] = lhs[1]        # fold A: res15
        w4[32 * g + 8: 32 * g + 16, 0:128] = lhs[0]   #         res14 x2
        w4[32 * g: 32 * g + 8, 128:256] = lhs[3]      # fold B: res17
        w4[32 * g + 8: 32 * g + 16, 128:256] = lhs[2]  #        res16 x2
        w4[32 * g: 32 * g + 8, 256:384] = lhs[6]      # final:  res20
        w4[32 * g + 8: 32 * g + 16, 256:384] = lhs[5]  #        res19 x2
        w4[32 * g + 16: 32 * g + 24, 256:384] = lhs[4]  #      res18 x4
    w4 = bf(w4)
    obias = np.ascontiguousarray(
        np.tile(np.asarray(inputs["out_bias"], np.float32), 4).reshape(128, 1))

    in_maps = []
    for q in range(8):
        pk = [_pack_res8(np.ascontiguousarray(r[q::8])) for r in res_levels]
        rhsab = bf(np.concatenate([
            np.concatenate([pk[1], np.tile(pk[0], (1, 2))], axis=0),
            np.concatenate([pk[3], np.tile(pk[2], (1, 2))], axis=0)], axis=1))
        rhsf = bf(np.concatenate(
            [pk[6], np.tile(pk[5], (1, 2)), np.tile(pk[4], (1, 4))], axis=0))
        in_maps.append({
            "rhsf": rhsf, "rhsab": rhsab,
            "o14": _pack_o_mod4(o_L0[q::8]),
            "w4": w4, "obias": obias,
        })
    return nc, in_maps


def assemble(results):
    full = np.empty((2 ** T, 32), np.float32)
    for q in range(8):
        full[q::8] = _unpack_o_mod4(results[q]["out"])
    return full.reshape((2,) * T + (32,))


def kernel(**inputs):
    nc, in_maps = prepare(inputs)
    res = run_bass_kernel_spmd(nc, in_maps, list(range(8)))
    return assemble(res.results)


# revision 7
# speedup vs baseline: 1.1654x; 1.0185x over previous
"""Trainium2 Bass kernel for nn_BinTreeNetwork (binary-tree MLP expansion).

Strategy (v3)
-------------
The reference is a 21-level binary-tree expansion ending at a (2,)^21 x 32
fp32 output (256 MB). Everything is linear; in flat memory terms each
iteration maps state rows (L, R: M x 2) and accumulator (out: M x 32) to
2M rows via

  res = [L[:M/2]; R[:M/2]; L[M/2:]; R[M/2:]]          (2M x 2)
  out' = [out + C[:M]; out + C[M:]],  C = res @ Wo_i.T
  L', R' = res @ Wl_i.T + bl_i,  res @ Wr_i.T + br_i

Mod-8 row sharding is communication-free (core q owns rows ≡ q mod 8) and
preserves the flat form. The host computes the tiny L/R state path exactly
in fp32 and ships only *res* planes (levels 14..20, bf16, wrap-replicated
where needed) plus the level-14 o-accumulator (fp32, 256 KB/core), so
per-core HBM traffic is ~37 MB and the 33.5 MB output write IS the
roofline (vs 52 MB/core for a ship-o20 variant).

Key devices tricks:
  * K-folding: because a matmul's cost is ~1 column/cycle regardless of K,
    multiple tree levels are contracted in ONE matmul by stacking their
    res planes on the K axis (lower levels pre-replicated by the host to
    match the wrap period).  Final pass: levels 18+19+20 as K=24.
    Internal: (14,15)->o16 and (16,17)->o18 as K=16.  The o-tree touches
    only 3 small fp32 SBUF tiles; o_19/o_20 never exist anywhere.
  * PE row-group rotation: successive chunks place weights+rhs at
    partition base 32g (g = chunk mod 4) with tile_position=(32g, 0), so
    up to 4 matmuls run concurrently in disjoint 32-row PE sub-arrays and
    per-chunk LDWEIGHTS is pulled ahead instead of serializing.
  * The fused out = PSUM + out_bias + o_18[wrapped] pass alternates
    between VectorE and GpSimdE per chunk - either alone would be the
    critical path.

o/out tiles use the "mod-4 stacked plane" layout [128, M/4]: partition
32*(row%4)+plane, column row//4, making the tree-doubling broadcast a pure
column-slice operation and all DMAs fully contiguous. The o-accumulator
path stays fp32 end to end; only res values and Wo weights are bf16
(measured ~1e-3 norm-rel, tolerance 2e-2).
"""
import numpy as np
import ml_dtypes
from contextlib import ExitStack

import concourse.bass as bass
import concourse.bacc as bacc
import concourse.mybir as mybir
import concourse.tile as tile
from concourse.bass_utils import run_bass_kernel_spmd

T = 21
L0 = 14          # host ships o at this level; device does levels 14..20
FCOLS = 65536    # final out cols per core ( 2^21/8 rows / 4 per col )
PAIR = 2048      # psum tile / out-DMA chunk cols (4 PSUM banks)
STCOLS = 8192    # streamed rhs tile cols (384 KB per input DMA)
BF16 = mybir.dt.bfloat16
F32 = mybir.dt.float32
ADD = mybir.AluOpType.add

# internal K=16 folds: (lo, hi) level pairs -> o_{hi+1}; final fold 18+19+20
FOLD_A_COLS = 2048    # C_15 cols; produces o_16
FOLD_B_COLS = 8192    # C_17 cols; produces o_18
O18C = 8192           # o_18 cols

_CACHE = {}


# ---------------- host-side exact precompute ----------------

def _host_precompute(inputs):
    x = inputs["x"].astype(np.float32)
    L = (x @ inputs["in_left_layer"].T + inputs["in_left_bias"]).reshape(1, 2).astype(np.float32)
    R = (x @ inputs["in_right_layer"].T + inputs["in_right_bias"]).reshape(1, 2).astype(np.float32)
    out = (x @ inputs["out_layer0"].T).reshape(1, 32).astype(np.float32)
    res_levels = []
    o_L0 = None
    for i in range(T):
        M = L.shape[0]
        if i == L0:
            o_L0 = out
        if M == 1:
            res = np.array([[L[0, 0], R[0, 0]], [L[0, 1], R[0, 1]]], np.float32)
        else:
            res = np.concatenate([L[: M // 2], R[: M // 2], L[M // 2 :], R[M // 2 :]], axis=0)
        if i >= L0:
            res_levels.append(res)
        if i < L0:
            C = res @ inputs["out_layers"][i].T
            out = np.concatenate([out + C[:M], out + C[M:]], axis=0)
        if i < T - 1:  # last level's L/R states are unused
            L = res @ inputs["tree_left_layers"][i].T + inputs["tree_left_biases"][i]
            R = res @ inputs["tree_right_layers"][i].T + inputs["tree_right_biases"][i]
    return o_L0, res_levels


def _pack_o_mod4(o_rows):
    M = o_rows.shape[0]
    return np.ascontiguousarray(
        o_rows.reshape(M // 4, 4, 32).transpose(1, 2, 0).reshape(128, M // 4), np.float32)


def _unpack_o_mod4(t):
    Mc = t.shape[1]
    return np.ascontiguousarray(
        t.reshape(4, 32, Mc).transpose(2, 0, 1).reshape(4 * Mc, 32), np.float32)


def _pack_res8(res):
    m2 = res.shape[0]
    cols = m2 // 4
    return np.ascontiguousarray(
        res.reshape(cols, 4, 2).transpose(1, 2, 0).reshape(8, cols), np.float32)


def _make_lhsT(Wo):
    t = np.zeros((8, 128), np.float32)
    for b in range(4):
        for f in range(2):
            t[2 * b + f, 32 * b: 32 * (b + 1)] = Wo[:, f]
    return t


# ---------------- device program ----------------

def _build_nc():
    nc = bacc.Bacc("TRN2", target_bir_lowering=False, debug=False,
                   enable_asserts=True, num_devices=8)

    rhsf_d = nc.dram_tensor("rhsf", [24, FCOLS], BF16, kind="ExternalInput").ap()
    rhsab_d = nc.dram_tensor("rhsab", [16, FOLD_A_COLS + FOLD_B_COLS], BF16,
                             kind="ExternalInput").ap()
    o14_d = nc.dram_tensor("o14", [128, 512], F32, kind="ExternalInput").ap()
    w4_d = nc.dram_tensor("w4", [128, 3 * 128], BF16, kind="ExternalInput").ap()
    obias_d = nc.dram_tensor("obias", [128, 1], F32, kind="ExternalInput").ap()
    out_d = nc.dram_tensor("out", [128, FCOLS], F32, kind="ExternalOutput").ap()

    with tile.TileContext(nc, trace_sim=False) as tc:
        ctx = ExitStack()
        with ctx:
            const_pool = ctx.enter_context(tc.tile_pool(name="consts", bufs=1))
            otree_pool = ctx.enter_context(tc.tile_pool(name="otree", bufs=1))
            rhs_pool = ctx.enter_context(tc.tile_pool(name="rhsc", bufs=5))
            outc_pool = ctx.enter_context(tc.tile_pool(name="outc", bufs=6))
            psum_pool = ctx.enter_context(tc.tile_pool(name="ps", bufs=2, space="PSUM"))

            w4_sb = const_pool.tile([128, 3 * 128], BF16, name="w4_sb")
            nc.scalar.dma_start(out=w4_sb[:], in_=w4_d[:])
            obias_sb = const_pool.tile([128, 1], F32, name="obias_sb")
            nc.scalar.dma_start(out=obias_sb[:], in_=obias_d[:])

            o14 = otree_pool.tile([128, 512], F32, name="o14_sb")
            o16 = otree_pool.tile([128, 2048], F32, name="o16_sb")
            o18 = otree_pool.tile([128, O18C], F32, name="o18_sb")
            nc.scalar.dma_start(out=o14[:], in_=o14_d[:])

            state = {"g": 0, "n": 0}

            def stream_tile(src_d, src_off, K, ncols):
                # one big input DMA; rhs lands at partition base 32g so the
                # matmuls of successive tiles hit disjoint PE row groups
                g = state["g"] % 4
                state["g"] += 1
                p0 = 32 * g
                st = rhs_pool.tile([128, ncols], BF16,
                                   name=f"st{state['g']}", tag="rhsc")
                nc.scalar.dma_start(out=st[p0:p0 + K, :ncols],
                                    in_=src_d[:, src_off: src_off + ncols])
                return st, p0

            def pair_mms(st, p0, K, wcol, coff):
                state["n"] += 1
                pt = psum_pool.tile([128, PAIR], F32, name=f"p{state['n']}", tag="ps")
                for s in range(0, PAIR, 512):
                    nc.tensor.matmul(pt[:, s:s + 512],
                                     w4_sb[p0:p0 + K, wcol:wcol + 128],
                                     st[p0:p0 + K, coff + s: coff + s + 512],
                                     start=True, stop=True, tile_position=(p0, 0))
                return pt

            # fold A: levels 14+15 -> o_16   (K=16, weights at w4 cols 0:128)
            stA, pA = stream_tile(rhsab_d, 0, 16, FOLD_A_COLS)
            pt = pair_mms(stA, pA, 16, 0, 0)
            for b in range(0, FOLD_A_COLS, 512):
                nc.vector.scalar_tensor_tensor(
                    o16[:, b:b + 512], pt[:, b:b + 512], 0.0,
                    o14[:, 0:512], ADD, ADD)

            # fold B: levels 16+17 -> o_18   (K=16, weights at w4 cols 128:256)
            stB = [stream_tile(rhsab_d, FOLD_A_COLS + 4096 * i, 16, 4096)
                   for i in (0, 1)]
            for jj in (0, 1):
                for i in (0, 1):
                    st, p0 = stB[i]
                    pt = pair_mms(st, p0, 16, 128, PAIR * jj)
                    a = 4096 * i + PAIR * jj
                    nc.vector.scalar_tensor_tensor(
                        o18[:, a:a + PAIR], pt[:], 0.0,
                        o16[:, 0:PAIR], ADD, ADD)

            # final: levels 18+19+20 -> out  (K=24, weights at w4 cols 256:384)
            for tp in range(0, FCOLS // STCOLS, 2):
                sts = [stream_tile(rhsf_d, STCOLS * (tp + i), 24, STCOLS)
                       for i in (0, 1)]
                for jj in range(STCOLS // PAIR):
                    for i in (0, 1):
                        st, p0 = sts[i]
                        pt = pair_mms(st, p0, 24, 256, PAIR * jj)
                        a = STCOLS * (tp + i) + PAIR * jj
                        otile = outc_pool.tile([128, PAIR], F32,
                                               name=f"ot{a}", tag="outc")
                        nc.vector.scalar_tensor_tensor(
                            otile[:], pt[:], obias_sb[:],
                            o18[:, PAIR * jj: PAIR * jj + PAIR], ADD, ADD)
                        nc.sync.dma_start(out=out_d[:, a: a + PAIR], in_=otile[:])

    nc.compile()
    return nc


# ---------------- entry point ----------------

def prepare(inputs):
    inputs = {k: np.asarray(v) for k, v in inputs.items()}
    o_L0, res_levels = _host_precompute(inputs)

    if "nc" not in _CACHE:
        _CACHE["nc"] = _build_nc()
    nc = _CACHE["nc"]

    bf = lambda a: np.ascontiguousarray(a).astype(ml_dtypes.bfloat16)
    lhs = [_make_lhsT(np.asarray(inputs["out_layers"][L0 + li], np.float32))
           for li in range(7)]   # levels 14..20 -> lhs[0..6]
    w4 = np.zeros((128, 3 * 128), np.float32)
    for g in range(4):
        w4[32 * g: 32 * g + 8, 0:128] = lhs[1]        # fold A: res15
        w4[32 * g + 8: 32 * g + 16, 0:128] = lhs[0]   #         res14 x2
        w4[32 * g: 32 * g + 8, 128:256] = lhs[3]      # fold B: res17
        w4[32 * g + 8: 32 * g + 16, 128:256] = lhs[2]  #        res16 x2
        w4[32 * g: 32 * g + 8, 256:384] = lhs[6]      # final:  res20
        w4[32 * g + 8: 32 * g + 16, 256:384] = lhs[5]  #        res19 x2
        w4[32 * g + 16: 32 * g + 24, 256:384] = lhs[4]  #      res18 x4
    w4 = bf(w4)
    obias = np.ascontiguousarray(
        np.tile(np.asarray(inputs["out_bias"], np.float32), 4).reshape(128, 1))

    in_maps = []
    for q in range(8):
        pk = [_pack_res8(np.ascontiguousarray(r[q::8])) for r in res_levels]
        rhsab = bf(np.concatenate([
            np.concatenate([pk[1], np.tile(pk[0], (1, 2))], axis=0),
            np.concatenate([pk[3], np.tile(pk[2], (1, 2))], axis=0)], axis=1))
        rhsf = bf(np.concatenate(
            [pk[6], np.tile(pk[5], (1, 2)), np.tile(pk[4], (1, 4))], axis=0))
        in_maps.append({
            "rhsf": rhsf, "rhsab": rhsab,
            "o14": _pack_o_mod4(o_L0[q::8]),
            "w4": w4, "obias": obias,
        })
    return nc, in_maps


def assemble(results):
    full = np.empty((2 ** T, 32), np.float32)
    for q in range(8):
        full[q::8] = _unpack_o_mod4(results[q]["out"])
    return full.reshape((2,) * T + (32,))


def kernel(**inputs):
    nc, in_maps = prepare(inputs)
    res = run_bass_kernel_spmd(nc, in_maps, list(range(8)))
    return assemble(res.results)


# revision 9
# speedup vs baseline: 1.3222x; 1.1345x over previous
"""Trainium2 Bass kernel for nn_BinTreeNetwork (binary-tree MLP expansion).

Strategy (v5)
-------------
The reference is a 21-level binary-tree expansion ending at a (2,)^21 x 32
fp32 output (256 MB). Everything is linear; in flat memory terms each
iteration maps state rows (L, R: M x 2) and accumulator (out: M x 32) to
2M rows via

  res = [L[:M/2]; R[:M/2]; L[M/2:]; R[M/2:]]          (2M x 2)
  out' = [out + C[:M]; out + C[M:]],  C = res @ Wo_i.T
  L', R' = res @ Wl_i.T + bl_i,  res @ Wr_i.T + br_i

Mod-8 row sharding is communication-free (core q owns rows ≡ q mod 8) and
preserves the flat form. The host computes the recursion exactly in fp32
through level 18 and ships, per core:
  * o18p [128, 8192] bf16 - the level-18 o-accumulator with out_bias
    pre-added (wrap period 8192 cols in the final output),
  * rhsf [24, 65536] bf16 - the level-18/19/20 res planes stacked on the
    contraction axis (lower levels wrap-replicated x2 / x4 by the host),
  * w4 [128, 128] bf16 - blockdiag lhsT for Wo20/Wo19/Wo18, replicated
    into all four 32-row PE groups.
Per-core HBM traffic is ~39 MB; the 33.5 MB output write is the roofline
(the ship-o20 baseline moved 52 MB/core).

Device: the entire remaining computation is, per 2048-col chunk,
  PSUM = blockdiag(Wo20|Wo19|Wo18).T @ rhsf-chunk     (one K=24 matmul
         per 512 cols - K-folding is free: matmul cost ~ columns only)
  out  = PSUM + o18p[wrapped]                          (fused add)
  DMA out-chunk -> DRAM
Chunks stream through two [24, 8192] rhs tiles whose partition base
rotates across the four PE row groups (tile_position=(32g, 0)), and the
PSUM drain alternates between VectorE (scalar_tensor_tensor) and
ScalarE-copy + GpSimdE-add so no single engine paces the loop; PSUM
ping-pongs in two 4-bank tiles.

o/out tiles use the "mod-4 stacked plane" layout [128, M/4]: partition
32*(row%4)+plane, column row//4, making the tree-doubling broadcast a pure
column-slice operation and all DMAs fully contiguous. res values, Wo
weights and the shipped o18 accumulator are bf16 (measured ~1.5e-3
norm-rel, tolerance 2e-2); PSUM accumulation and the output are fp32.
"""
import numpy as np
import ml_dtypes
from contextlib import ExitStack

import concourse.bass as bass
import concourse.bacc as bacc
import concourse.mybir as mybir
import concourse.tile as tile
from concourse.bass_utils import run_bass_kernel_spmd

T = 21
L0 = 18          # host ships o at this level; device does levels 18..20
FCOLS = 65536    # final out cols per core ( 2^21/8 rows / 4 per col )
O18C = 8192      # o_18 cols per core (wrap period of the o18p operand)
PAIR = 2048      # psum tile / out-DMA chunk cols (4 PSUM banks)
STCOLS = 8192    # streamed rhs tile cols (384 KB per input DMA)
BF16 = mybir.dt.bfloat16
F32 = mybir.dt.float32
ADD = mybir.AluOpType.add

_CACHE = {}


# ---------------- host-side exact precompute ----------------

def _host_precompute(inputs):
    x = inputs["x"].astype(np.float32)
    L = (x @ inputs["in_left_layer"].T + inputs["in_left_bias"]).reshape(1, 2).astype(np.float32)
    R = (x @ inputs["in_right_layer"].T + inputs["in_right_bias"]).reshape(1, 2).astype(np.float32)
    out = (x @ inputs["out_layer0"].T).reshape(1, 32).astype(np.float32)
    res_levels = []
    o_L0 = None
    for i in range(T):
        M = L.shape[0]
        if i == L0:
            o_L0 = out
        if M == 1:
            res = np.array([[L[0, 0], R[0, 0]], [L[0, 1], R[0, 1]]], np.float32)
        else:
            res = np.concatenate([L[: M // 2], R[: M // 2], L[M // 2 :], R[M // 2 :]], axis=0)
        if i >= L0:
            res_levels.append(res)
        if i < L0:
            C = res @ inputs["out_layers"][i].T
            out = np.concatenate([out + C[:M], out + C[M:]], axis=0)
        if i < T - 1:  # last level's L/R states are unused
            L = res @ inputs["tree_left_layers"][i].T + inputs["tree_left_biases"][i]
            R = res @ inputs["tree_right_layers"][i].T + inputs["tree_right_biases"][i]
    return o_L0, res_levels


def _pack_o_mod4(o_rows):
    M = o_rows.shape[0]
    return np.ascontiguousarray(
        o_rows.reshape(M // 4, 4, 32).transpose(1, 2, 0).reshape(128, M // 4), np.float32)


def _unpack_o_mod4(t):
    Mc = t.shape[1]
    return np.ascontiguousarray(
        t.reshape(4, 32, Mc).transpose(2, 0, 1).reshape(4 * Mc, 32), np.float32)


def _pack_res8(res):
    m2 = res.shape[0]
    cols = m2 // 4
    return np.ascontiguousarray(
        res.reshape(cols, 4, 2).transpose(1, 2, 0).reshape(8, cols), np.float32)


def _make_lhsT(Wo):
    t = np.zeros((8, 128), np.float32)
    for b in range(4):
        for f in range(2):
            t[2 * b + f, 32 * b: 32 * (b + 1)] = Wo[:, f]
    return t


# ---------------- device program ----------------

def _build_nc():
    nc = bacc.Bacc("TRN2", target_bir_lowering=False, debug=False,
                   enable_asserts=True, num_devices=8)

    rhsf_d = nc.dram_tensor("rhsf", [24, FCOLS], BF16, kind="ExternalInput").ap()
    o18_d = nc.dram_tensor("o18p", [128, O18C], BF16, kind="ExternalInput").ap()
    w4_d = nc.dram_tensor("w4", [128, 128], BF16, kind="ExternalInput").ap()
    out_d = nc.dram_tensor("out", [128, FCOLS], F32, kind="ExternalOutput").ap()

    with tile.TileContext(nc, trace_sim=False) as tc:
        ctx = ExitStack()
        with ctx:
            const_pool = ctx.enter_context(tc.tile_pool(name="consts", bufs=1))
            rhs_pool = ctx.enter_context(tc.tile_pool(name="rhsc", bufs=5))
            outc_pool = ctx.enter_context(tc.tile_pool(name="outc", bufs=6))
            psum_pool = ctx.enter_context(tc.tile_pool(name="ps", bufs=2, space="PSUM"))

            w4_sb = const_pool.tile([128, 128], BF16, name="w4_sb")
            nc.gpsimd.dma_start(out=w4_sb[:], in_=w4_d[:])
            o18p = const_pool.tile([128, O18C], BF16, name="o18p_sb")
            # quarters so the first pairs aren't gated on the full 2 MB
            for qq in range(4):
                nc.gpsimd.dma_start(out=o18p[:, PAIR * qq: PAIR * (qq + 1)],
                                    in_=o18_d[:, PAIR * qq: PAIR * (qq + 1)])

            state = {"g": 0, "n": 0}

            def stream_tile(src_off):
                # one big input DMA; rhs lands at partition base 32g so the
                # matmuls of successive tiles hit disjoint PE row groups
                g = state["g"] % 4
                state["g"] += 1
                p0 = 32 * g
                st = rhs_pool.tile([128, STCOLS], BF16,
                                   name=f"st{state['g']}", tag="rhsc")
                nc.scalar.dma_start(out=st[p0:p0 + 24, :],
                                    in_=rhsf_d[:, src_off: src_off + STCOLS])
                return st, p0

            def pair_mms(st, p0, coff):
                state["n"] += 1
                pt = psum_pool.tile([128, PAIR], F32, name=f"p{state['n']}", tag="ps")
                for s in range(0, PAIR, 512):
                    nc.tensor.matmul(pt[:, s:s + 512],
                                     w4_sb[p0:p0 + 24, :],
                                     st[p0:p0 + 24, coff + s: coff + s + 512],
                                     start=True, stop=True, tile_position=(p0, 0))
                return pt

            for tp in range(0, FCOLS // STCOLS, 2):
                sts = [stream_tile(STCOLS * (tp + i)) for i in (0, 1)]
                for jj in range(STCOLS // PAIR):
                    osl = o18p[:, PAIR * jj: PAIR * (jj + 1)]
                    for i in (0, 1):
                        st, p0 = sts[i]
                        pt = pair_mms(st, p0, PAIR * jj)
                        a = STCOLS * (tp + i) + PAIR * jj
                        otile = outc_pool.tile([128, PAIR], F32,
                                               name=f"ot{a}", tag="outc")
                        if i == 0:
                            nc.vector.scalar_tensor_tensor(
                                otile[:], pt[:], 0.0, osl, ADD, ADD)
                        else:
                            nc.scalar.copy(otile[:], pt[:])
                            nc.gpsimd.tensor_add(otile[:], otile[:], osl)
                        nc.sync.dma_start(out=out_d[:, a: a + PAIR], in_=otile[:])

    nc.compile()
    return nc


# ---------------- entry point ----------------

def prepare(inputs):
    inputs = {k: np.asarray(v) for k, v in inputs.items()}
    o_L0, res_levels = _host_precompute(inputs)

    if "nc" not in _CACHE:
        _CACHE["nc"] = _build_nc()
    nc = _CACHE["nc"]

    bf = lambda a: np.ascontiguousarray(a).astype(ml_dtypes.bfloat16)
    lhs = {i: _make_lhsT(np.asarray(inputs["out_layers"][i], np.float32))
           for i in (18, 19, 20)}
    w4 = np.zeros((128, 128), np.float32)
    for g in range(4):
        w4[32 * g: 32 * g + 8] = lhs[20]
        w4[32 * g + 8: 32 * g + 16] = lhs[19]
        w4[32 * g + 16: 32 * g + 24] = lhs[18]
    w4 = bf(w4)
    obias_col = np.tile(np.asarray(inputs["out_bias"], np.float32), 4).reshape(128, 1)

    in_maps = []
    for q in range(8):
        pk = [_pack_res8(np.ascontiguousarray(r[q::8])) for r in res_levels]
        rhsf = bf(np.concatenate(
            [pk[2], np.tile(pk[1], (1, 2)), np.tile(pk[0], (1, 4))], axis=0))
        o18p = bf(_pack_o_mod4(o_L0[q::8]) + obias_col)
        in_maps.append({"rhsf": rhsf, "o18p": o18p, "w4": w4})
    return nc, in_maps


def assemble(results):
    full = np.empty((2 ** T, 32), np.float32)
    for q in range(8):
        full[q::8] = _unpack_o_mod4(results[q]["out"])
    return full.reshape((2,) * T + (32,))


def kernel(**inputs):
    nc, in_maps = prepare(inputs)
    res = run_bass_kernel_spmd(nc, in_maps, list(range(8)))
    return assemble(res.results)
